# revision 7
# baseline (speedup 1.0000x reference)
"""TRN2 Bass kernel: fused LSTM cell (nn_CustomLSTMCell), 8-core tensor-parallel.

Strategy
--------
gates = x @ W_ih.T + b_ih + h_prev @ W_hh.T + b_hh  is computed as ONE GEMM
with contraction K = I + H = 4096 over xh = [x | h_prev] and W = [W_ih | W_hh].

The 4H gate dimension is tensor-parallel sharded across the 8 cores: core c
owns h-columns [c*256, (c+1)*256) of every gate (i, f, g, o).  Each core
computes gatesT [1024, 2048] = Wc @ xh.T with gate rows on partitions, so the
per-gate bias is a native per-partition scalar in scalar.activation, which
also applies sigmoid/tanh while evicting PSUM -> SBUF.  The LSTM cell update
(new_C = f*C + i*g, new_h = o*tanh(new_C)) runs on the vector engine, fully
overlapped with the tensor engine.  No collectives: output slices are
disjoint and gathered on the host.

Matmul operands are cast to fp16 on the host (halves DMA traffic, 4x PE rate
vs fp32, ~8x more mantissa than bf16); accumulation stays fp32 in PSUM and
the epilogue is fp32.

Loop shape: all 8 PSUM banks (2 h-blocks x 4 gates) accumulate
simultaneously with the k-chunk loop OUTER, so one 384KB chunk of
(w, xh) feeds 8 matmuls -> the DMA demand rate of the first batch-tile is
~220GB/s, below what the DMA rings sustain (~2 HWDGE rings x ~170GB/s);
the old per-h-block loop demanded ~300GB/s and starved the PE early, which
also dropped the HAM clock to 1.2GHz for several us.  The last few chunks
run per-bank (staggered tail) so PSUM evictions pipeline and the next
group's bank reuse never stalls.

DMA plan: few, large transfers with multi-KB contiguous per-partition lines
(host pre-arranges every tensor partition-major).  w streams in 1MB blocks
alternating between the Sync HWDGE and GpSimd SWDGE queues, xh generation 0
on the Scalar HWDGE queue, later generations on whichever ring is free, and
each group's six outputs are packed into one [128, 3072] tile stored with a
single DMA.
"""

import numpy as np

B = 2048           # batch
I_DIM = 2048       # input features
H = 2048           # hidden
NCORES = 8
S = H // NCORES    # 256: per-core h-slice (per gate)
M_PER_CORE = 4 * S # 1024 gate rows per core
K = I_DIM + H      # 4096 fused contraction dim
P = 128
KC = K // P        # 32 contraction chunks
NT = B // 512      # 4 batch tiles of 512
HB = S // P        # 2 h-blocks of 128 per core
NQ = 6             # packed outputs per group: f,i,g,cn,o,h

_BF16 = np.float16

_CACHE = {}

# k-chunk DMA blocks, in consumption order on a single ring each: first
# block small so the PE's first real matmul starts early; later blocks big
# for DMA efficiency.  Sized so each block completes (at the ~170GB/s one
# ring sustains when two rings are active) >=3us before the PE needs its
# first chunk at the fused-loop rate of 1.73us/chunk.
_XBLOCKS = [(0, 2), (2, 8), (8, 16), (16, 32)]
_WBLOCKS = [(0, 2), (2, 6), (6, 12), (12, 20), (20, 32)]


def _build_program():
    from contextlib import ExitStack

    import concourse.mybir as mybir
    import concourse.tile as tile
    from concourse import bacc

    f32 = mybir.dt.float32
    bf16 = mybir.dt.float16
    AF = mybir.ActivationFunctionType

    nc = bacc.Bacc("TRN2", target_bir_lowering=False, debug=False)

    # Host layouts are partition-major so every DMA moves multi-KB
    # contiguous lines per partition.
    w_d = nc.dram_tensor("w_t", [P * KC, 8 * P], bf16, kind="ExternalInput").ap()
    xh_d = [
        nc.dram_tensor(f"xh{n}", [P * KC, 512], bf16, kind="ExternalInput").ap()
        for n in range(NT)
    ]
    bias_d = nc.dram_tensor("bias", [P, 4 * HB], f32, kind="ExternalInput").ap()
    c_d = nc.dram_tensor("c_t", [P * HB, B], f32, kind="ExternalInput").ap()
    out_d = nc.dram_tensor(
        "out", [NT * HB * P, NQ * 512], f32, kind="ExternalOutput"
    ).ap()

    w_r = w_d.rearrange("(p a) m -> p a m", p=P)         # [128, 32, 1024]
    xh_r = [t.rearrange("(p a) m -> p a m", p=P) for t in xh_d]
    c_r = c_d.rearrange("(p h) m -> p h m", p=P)         # [128, 2, 2048]
    out_r = out_d.rearrange("(g p) m -> g p m", g=NT * HB)  # [8, 128, 3072]

    # gate order within a group: i, f, g, o (torch chunk order); packed
    # output column order: f, i, g, cn, o, h (early finishers first so the
    # final group's tail store can fire before o/h are ready).
    ACT_FN = [AF.Sigmoid, AF.Sigmoid, AF.Tanh, AF.Sigmoid]
    QCOL = {"i": 1, "f": 0, "g": 2, "cn": 3, "o": 4, "h": 5}
    QN = ["i", "f", "g", "o"]

    with tile.TileContext(nc) as tc, ExitStack() as ctx:
        w_pool = ctx.enter_context(tc.tile_pool(name="w", bufs=1))
        xh_pool = ctx.enter_context(tc.tile_pool(name="xh", bufs=2))
        c_pool = ctx.enter_context(tc.tile_pool(name="c", bufs=1))
        b_pool = ctx.enter_context(tc.tile_pool(name="b", bufs=1))
        psum_pool = ctx.enter_context(tc.tile_pool(name="ps", bufs=1, space="PSUM"))
        out_pool = ctx.enter_context(tc.tile_pool(name="out", bufs=2))
        tmp_pool = ctx.enter_context(tc.tile_pool(name="tmp", bufs=2))

        # Bias rides the (otherwise idle-early) GpSimd queue so it never
        # delays the critical w/xh streams; it's only needed at ~60us.
        bias_all = b_pool.tile([P, 4 * HB], f32)
        nc.gpsimd.dma_start(bias_all[:], bias_d[:, :])

        # A few matmuls on dummy data bridge the framework-preamble gap so
        # HAM warm-up overlaps the first block DMAs.  Never read.
        dummy = b_pool.tile([P, 512], bf16)
        nc.vector.memset(dummy[:], 0.0)
        warm_ps = psum_pool.tile([P, 512], f32, name="ps00")
        NWARM = 10
        for i in range(NWARM):
            nc.tensor.matmul(
                warm_ps[:], dummy[:, 0:P], dummy[:],
                start=(i == 0), stop=(i == NWARM - 1),
            )

        # Input streams, one ring per stream, each in strict consumption
        # order (concurrent rings split the ~340GB/s fair-share by packet,
        # so putting later-needed data on the ring ahead of earlier-needed
        # data starves the PE).  Sync: w blocks ascending.  Scalar: xh0,
        # then C (first epilogue ~60us), then xh1 (group 1 starts ~66us).
        # GpSimd picks up xh2/xh3 inside the loop (their buffer-reuse waits
        # must not block a queue with timely work).
        w_sb = w_pool.tile([P, KC, 8 * P], bf16)
        for a0, a1 in _WBLOCKS:
            nc.sync.dma_start(w_sb[:, a0:a1, :], w_r[:, a0:a1, :])

        def alloc_xh():
            return xh_pool.tile([P, KC, 512], bf16, name="xhg")

        xh_tiles = {0: alloc_xh(), 1: alloc_xh()}
        for a0, a1 in _XBLOCKS:
            nc.scalar.dma_start(xh_tiles[0][:, a0:a1, :], xh_r[0][:, a0:a1, :])

        c_all = c_pool.tile([P, HB, B], f32)
        nc.scalar.dma_start(c_all[:], c_r[:, :, :])

        nc.scalar.dma_start(xh_tiles[1][:, 0:16, :], xh_r[1][:, 0:16, :])
        nc.scalar.dma_start(xh_tiles[1][:, 16:KC, :], xh_r[1][:, 16:KC, :])

        def mm(ps_t, hb, g, k, **kw):
            m0 = (hb * 4 + g) * P
            nc.tensor.matmul(
                ps_t[:], w_sb[:, k, m0 : m0 + P], xh[:, k, :], **kw
            )

        for n in range(NT):
            ns = slice(n * 512, (n + 1) * 512)
            final = n == NT - 1
            # prefetch the n+1 generation one iteration ahead on GpSimd
            # (slot frees when the n-1 generation's readers finish)
            if n >= 1 and n + 1 < NT:
                xh_tiles[n + 1] = alloc_xh()
                nc.gpsimd.dma_start(
                    xh_tiles[n + 1][:, 0:16, :], xh_r[n + 1][:, 0:16, :]
                )
                nc.gpsimd.dma_start(
                    xh_tiles[n + 1][:, 16:KC, :], xh_r[n + 1][:, 16:KC, :]
                )
            xh = xh_tiles[n]

            ps = {
                (hb, g): psum_pool.tile([P, 512], f32, name=f"ps{hb}{g}")
                for hb in range(HB)
                for g in range(4)
            }
            tail = 8 if final else 4
            split = KC - tail
            for k in range(split):
                for hb in range(HB):
                    for g in range(4):
                        mm(ps[hb, g], hb, g, k, start=(k == 0), stop=False)

            # staggered tail: each bank finishes its last chunks as a
            # contiguous run, so evictions (and the epilogues) pipeline
            # against the remaining banks' matmuls.  For the final group
            # the f,i,g banks of hb1 close before o, so the cell-state
            # chain overlaps the o matmuls and the post-MM tail is just
            # o-sigmoid -> h-mul -> store.
            for hb in range(HB):
                gate_order = (1, 0, 2, 3) if final else (0, 1, 2, 3)
                gi = n * HB + hb
                ot = out_pool.tile([P, NQ * 512], f32, name="og")

                def q(name):
                    j = QCOL[name]
                    return ot[:, j * 512 : (j + 1) * 512]

                def gate_act(g):
                    m = hb * 4 + g
                    nc.scalar.activation(
                        q(QN[g]), ps[hb, g][:], ACT_FN[g],
                        bias=bias_all[:, m : m + 1],
                    )

                last_g = gate_order[3]
                for g in gate_order[:3]:
                    for k in range(split, KC):
                        mm(ps[hb, g], hb, g, k, start=False, stop=(k == KC - 1))
                if final:
                    # acts f,i,g and the cell-state chain overlap the o
                    # matmul tail; only o-sigmoid + h-mul remain after it.
                    for g in gate_order[:3]:
                        gate_act(g)
                    for k in range(split, KC):
                        mm(ps[hb, last_g], hb, last_g, k,
                           start=False, stop=(k == KC - 1))
                else:
                    # o-act is emitted before the (cn-gated) tanh so the
                    # o PSUM bank frees promptly for the next group.
                    for k in range(split, KC):
                        mm(ps[hb, last_g], hb, last_g, k,
                           start=False, stop=(k == KC - 1))
                    for g in gate_order[:3]:
                        gate_act(g)
                    gate_act(3)

                fc = tmp_pool.tile([P, 512], f32, name="fc")
                nc.vector.tensor_mul(fc[:], q("f"), c_all[:, hb, ns])
                ig = tmp_pool.tile([P, 512], f32, name="ig")
                nc.vector.tensor_mul(ig[:], q("i"), q("g"))
                nc.vector.tensor_add(q("cn"), ig[:], fc[:])
                th = tmp_pool.tile([P, 512], f32, name="th")
                nc.scalar.activation(th[:], q("cn"), AF.Tanh)
                if final:
                    gate_act(3)  # o
                nc.vector.tensor_mul(q("h"), q("o"), th[:])

                if final and hb == HB - 1:
                    # f,i,g,cn are done before the o matmul block ends:
                    # store them early on the (idle) Sync queue; the tail
                    # after the last MM is just o-store + h-store.
                    nc.sync.dma_start(out_r[gi][:, 0 : 4 * 512], ot[:, 0 : 4 * 512])
                    nc.scalar.dma_start(
                        out_r[gi][:, 4 * 512 : 5 * 512], ot[:, 4 * 512 : 5 * 512]
                    )
                    nc.scalar.dma_start(
                        out_r[gi][:, 5 * 512 : 6 * 512], ot[:, 5 * 512 : 6 * 512]
                    )
                else:
                    nc.sync.dma_start(out_r[gi][:, :], ot[:])

    nc.compile()
    return nc


def _get_program():
    if "nc" not in _CACHE:
        _CACHE["nc"] = _build_program()
    return _CACHE["nc"]


def _gate_row_index(core: int) -> np.ndarray:
    """Global rows of W/b (4H-dim) owned by `core`, in [hb][gate][r] order."""
    idx = []
    for hb in range(HB):
        for g in range(4):
            base = g * H + core * S + hb * P
            idx.extend(range(base, base + P))
    return np.asarray(idx)


def _part_major(a2d: np.ndarray) -> np.ndarray:
    """[K, M] -> [(p a), M] rows ordered p-major (row = p*KC + a)."""
    k, m = a2d.shape
    assert k == P * KC
    return np.ascontiguousarray(
        a2d.reshape(KC, P, m).transpose(1, 0, 2).reshape(k, m)
    )


def kernel(x, h_prev, C_prev, W_ih, b_ih, W_hh, b_hh):
    from concourse.bass_utils import run_bass_kernel_spmd

    nc = _get_program()

    xh_full = np.concatenate([x, h_prev], axis=1).T.astype(_BF16)  # [4096, 2048]
    xh_gens = [
        _part_major(np.ascontiguousarray(xh_full[:, n * 512 : (n + 1) * 512]))
        for n in range(NT)
    ]
    bias_full = (b_ih + b_hh).astype(np.float32)

    in_maps = []
    for c in range(NCORES):
        idx = _gate_row_index(c)
        w_cat = np.concatenate([W_ih[idx], W_hh[idx]], axis=1).astype(_BF16)
        cs = C_prev[:, c * S : (c + 1) * S].T  # [256, 2048]
        in_map = {
            "w_t": _part_major(np.ascontiguousarray(w_cat.T)),  # [4096, 1024]
            "bias": np.ascontiguousarray(bias_full[idx].reshape(4 * HB, P).T),
            "c_t": np.ascontiguousarray(
                cs.reshape(HB, P, B).transpose(1, 0, 2).reshape(HB * P, B)
            ),
        }
        for n in range(NT):
            in_map[f"xh{n}"] = xh_gens[n]
        in_maps.append(in_map)

    _CACHE["last_in_maps"] = in_maps
    res = run_bass_kernel_spmd(nc, in_maps, core_ids=list(range(NCORES)))

    # res.results[c]["out"]: [8*128, 3072] -> [n, hb, p, q, col]
    QNAMES = ["f_t", "i_t", "g_t", "cn_t", "o_t", "h_t"]
    full = {}
    parts = [
        res.results[c]["out"].reshape(NT, HB, P, NQ, 512) for c in range(NCORES)
    ]
    for qi, qn in enumerate(QNAMES):
        # rows: core-major h index (c, hb, p); cols: (n, col)
        t = np.concatenate(
            [
                parts[c][:, :, :, qi, :]
                .transpose(1, 2, 0, 3)
                .reshape(S, B)
                for c in range(NCORES)
            ],
            axis=0,
        )  # [H, B]
        full[qn] = np.ascontiguousarray(t.T)

    return (
        full["h_t"],
        full["cn_t"],
        full["f_t"],
        full["i_t"],
        full["g_t"],
        full["o_t"],
    )


# revision 11
# speedup vs baseline: 1.0185x; 1.0185x over previous
"""TRN2 Bass kernel: fused LSTM cell (nn_CustomLSTMCell), 8-core tensor-parallel.

Strategy
--------
gates = x @ W_ih.T + b_ih + h_prev @ W_hh.T + b_hh  is computed as ONE GEMM
with contraction K = I + H = 4096 over xh = [x | h_prev] and W = [W_ih | W_hh].

The 4H gate dimension is tensor-parallel sharded across the 8 cores: core c
owns h-columns [c*256, (c+1)*256) of every gate (i, f, g, o).  Each core
computes gatesT [1024, 2048] = Wc @ xh.T with gate rows on partitions, so the
per-gate bias is a native per-partition scalar in scalar.activation, which
also applies sigmoid/tanh while evicting PSUM -> SBUF.  The LSTM cell update
(new_C = f*C + i*g, new_h = o*tanh(new_C)) runs on the vector engine, fully
overlapped with the tensor engine.  No collectives: output slices are
disjoint and gathered on the host.

Matmul operands are cast to fp16 on the host (halves DMA traffic, 4x PE rate
vs fp32, ~8x more mantissa than bf16); accumulation stays fp32 in PSUM and
the epilogue is fp32.

Loop shape: all 8 PSUM banks (2 h-blocks x 4 gates) accumulate
simultaneously with the k-chunk loop OUTER, so one 384KB chunk of
(w, xh) feeds 8 matmuls -> the DMA demand rate of the first batch-tile is
~220GB/s, below what the DMA rings sustain (~2 HWDGE rings x ~170GB/s);
the old per-h-block loop demanded ~300GB/s and starved the PE early, which
also dropped the HAM clock to 1.2GHz for several us.  The last few chunks
run per-bank (staggered tail) so PSUM evictions pipeline and the next
group's bank reuse never stalls.

DMA plan: few, large transfers with multi-KB contiguous per-partition lines
(host pre-arranges every tensor partition-major).  w streams in 1MB blocks
alternating between the Sync HWDGE and GpSimd SWDGE queues, xh generation 0
on the Scalar HWDGE queue, later generations on whichever ring is free, and
each group's six outputs are packed into one [128, 3072] tile stored with a
single DMA.
"""

import numpy as np

B = 2048           # batch
I_DIM = 2048       # input features
H = 2048           # hidden
NCORES = 8
S = H // NCORES    # 256: per-core h-slice (per gate)
M_PER_CORE = 4 * S # 1024 gate rows per core
K = I_DIM + H      # 4096 fused contraction dim
P = 128
KC = K // P        # 32 contraction chunks
NT = B // 512      # 4 batch tiles of 512
HB = S // P        # 2 h-blocks of 128 per core
NQ = 6             # packed outputs per group: f,i,g,cn,o,h

_BF16 = np.float16

_CACHE = {}

# k-chunk DMA blocks, in consumption order on a single ring each: first
# block small so the PE's first real matmul starts early; later blocks big
# for DMA efficiency.  Sized so each block completes (at the ~170GB/s one
# ring sustains when two rings are active) >=3us before the PE needs its
# first chunk at the fused-loop rate of 1.73us/chunk.
_XBLOCKS = [(0, 2), (2, 8), (8, 16), (16, 32)]
_WBLOCKS = [(0, 2), (2, 6), (6, 12), (12, 20), (20, 32)]


def _build_program():
    from contextlib import ExitStack

    import concourse.mybir as mybir
    import concourse.tile as tile
    from concourse import bacc

    f32 = mybir.dt.float32
    bf16 = mybir.dt.float16
    AF = mybir.ActivationFunctionType

    nc = bacc.Bacc("TRN2", target_bir_lowering=False, debug=False)

    # Host layouts are partition-major so every DMA moves multi-KB
    # contiguous lines per partition.
    w_d = nc.dram_tensor("w_t", [P * KC, 8 * P], bf16, kind="ExternalInput").ap()
    xh_d = [
        nc.dram_tensor(f"xh{n}", [P * KC, 512], bf16, kind="ExternalInput").ap()
        for n in range(NT)
    ]
    bias_d = nc.dram_tensor("bias", [P, 4 * HB], f32, kind="ExternalInput").ap()
    c_d = nc.dram_tensor("c_t", [P * HB, B], f32, kind="ExternalInput").ap()
    out_d = nc.dram_tensor(
        "out", [NT * HB * P, NQ * 512], f32, kind="ExternalOutput"
    ).ap()

    w_r = w_d.rearrange("(p a) m -> p a m", p=P)         # [128, 32, 1024]
    xh_r = [t.rearrange("(p a) m -> p a m", p=P) for t in xh_d]
    c_r = c_d.rearrange("(p h) m -> p h m", p=P)         # [128, 2, 2048]
    out_r = out_d.rearrange("(g p) m -> g p m", g=NT * HB)  # [8, 128, 3072]

    # gate order within a group: i, f, g, o (torch chunk order); packed
    # output column order: g, i, f, cn, o, h (early finishers first so the
    # final group's tail stores can fire before o/h are ready).
    ACT_FN = [AF.Sigmoid, AF.Sigmoid, AF.Tanh, AF.Sigmoid]
    QCOL = {"g": 0, "i": 1, "f": 2, "cn": 3, "o": 4, "h": 5}
    QN = ["i", "f", "g", "o"]

    with tile.TileContext(nc) as tc, ExitStack() as ctx:
        w_pool = ctx.enter_context(tc.tile_pool(name="w", bufs=1))
        xh_pool = ctx.enter_context(tc.tile_pool(name="xh", bufs=2))
        c_pool = ctx.enter_context(tc.tile_pool(name="c", bufs=1))
        b_pool = ctx.enter_context(tc.tile_pool(name="b", bufs=1))
        psum_pool = ctx.enter_context(tc.tile_pool(name="ps", bufs=1, space="PSUM"))
        out_pool = ctx.enter_context(tc.tile_pool(name="out", bufs=2))
        tmp_pool = ctx.enter_context(tc.tile_pool(name="tmp", bufs=2))

        # Bias rides the (otherwise idle-early) GpSimd queue so it never
        # delays the critical w/xh streams; it's only needed at ~60us.
        bias_all = b_pool.tile([P, 4 * HB], f32)
        nc.gpsimd.dma_start(bias_all[:], bias_d[:, :])

        # A few matmuls on dummy data bridge the framework-preamble gap so
        # HAM warm-up overlaps the first block DMAs.  Never read.
        dummy = b_pool.tile([P, 512], bf16)
        nc.vector.memset(dummy[:], 0.0)
        warm_ps = psum_pool.tile([P, 512], f32, name="ps00")
        NWARM = 10
        for i in range(NWARM):
            nc.tensor.matmul(
                warm_ps[:], dummy[:, 0:P], dummy[:],
                start=(i == 0), stop=(i == NWARM - 1),
            )

        # Input streams, one ring per stream, each in strict consumption
        # order (concurrent rings split the ~340GB/s fair-share by packet,
        # so putting later-needed data on the ring ahead of earlier-needed
        # data starves the PE).  Sync: w blocks ascending.  Scalar: xh0,
        # then C (first epilogue ~60us), then xh1 (group 1 starts ~66us).
        # GpSimd picks up xh2/xh3 inside the loop (their buffer-reuse waits
        # must not block a queue with timely work).
        w_sb = w_pool.tile([P, KC, 8 * P], bf16)
        for a0, a1 in _WBLOCKS:
            nc.sync.dma_start(w_sb[:, a0:a1, :], w_r[:, a0:a1, :])

        def alloc_xh():
            return xh_pool.tile([P, KC, 512], bf16, name="xhg")

        xh_tiles = {0: alloc_xh(), 1: alloc_xh()}
        for a0, a1 in _XBLOCKS:
            nc.scalar.dma_start(xh_tiles[0][:, a0:a1, :], xh_r[0][:, a0:a1, :])

        c_all = c_pool.tile([P, HB, B], f32)
        nc.scalar.dma_start(c_all[:], c_r[:, :, :])

        nc.scalar.dma_start(xh_tiles[1][:, 0:16, :], xh_r[1][:, 0:16, :])
        nc.scalar.dma_start(xh_tiles[1][:, 16:KC, :], xh_r[1][:, 16:KC, :])

        def mm(ps_t, hb, g, k, **kw):
            m0 = (hb * 4 + g) * P
            nc.tensor.matmul(
                ps_t[:], w_sb[:, k, m0 : m0 + P], xh[:, k, :], **kw
            )

        for n in range(NT):
            ns = slice(n * 512, (n + 1) * 512)
            final = n == NT - 1
            # prefetch the n+1 generation one iteration ahead on GpSimd
            # (slot frees when the n-1 generation's readers finish)
            if n >= 1 and n + 1 < NT:
                xh_tiles[n + 1] = alloc_xh()
                nc.gpsimd.dma_start(
                    xh_tiles[n + 1][:, 0:16, :], xh_r[n + 1][:, 0:16, :]
                )
                nc.gpsimd.dma_start(
                    xh_tiles[n + 1][:, 16:KC, :], xh_r[n + 1][:, 16:KC, :]
                )
            xh = xh_tiles[n]

            ps = {
                (hb, g): psum_pool.tile([P, 512], f32, name=f"ps{hb}{g}")
                for hb in range(HB)
                for g in range(4)
            }
            tail = 8 if final else 4
            split = KC - tail
            for k in range(split):
                for hb in range(HB):
                    for g in range(4):
                        mm(ps[hb, g], hb, g, k, start=(k == 0), stop=False)

            # staggered tail: each bank finishes its last chunks as a
            # contiguous run, so evictions (and the epilogues) pipeline
            # against the remaining banks' matmuls.  For the final group
            # the g,i,f banks of hb1 close before o (g first: the i*g and
            # f*C products gate the tanh), so the whole cell-state chain
            # completes during the o matmuls and the post-MM tail is just
            # o-sigmoid -> h-mul -> store.
            for hb in range(HB):
                gate_order = (2, 0, 1, 3) if final else (0, 1, 2, 3)
                gi = n * HB + hb
                ot = out_pool.tile([P, NQ * 512], f32, name="og")

                def q(name):
                    j = QCOL[name]
                    return ot[:, j * 512 : (j + 1) * 512]

                def gate_act(g):
                    m = hb * 4 + g
                    nc.scalar.activation(
                        q(QN[g]), ps[hb, g][:], ACT_FN[g],
                        bias=bias_all[:, m : m + 1],
                    )

                last_g = gate_order[3]
                for g in gate_order[:3]:
                    for k in range(split, KC):
                        mm(ps[hb, g], hb, g, k, start=False, stop=(k == KC - 1))
                if final:
                    # acts f,i,g and the cell-state chain overlap the o
                    # matmul tail; only o-sigmoid + h-mul remain after it.
                    for g in gate_order[:3]:
                        gate_act(g)
                    for k in range(split, KC):
                        mm(ps[hb, last_g], hb, last_g, k,
                           start=False, stop=(k == KC - 1))
                else:
                    # o-act is emitted before the (cn-gated) tanh so the
                    # o PSUM bank frees promptly for the next group.
                    for k in range(split, KC):
                        mm(ps[hb, last_g], hb, last_g, k,
                           start=False, stop=(k == KC - 1))
                    for g in gate_order[:3]:
                        gate_act(g)
                    gate_act(3)

                fc = tmp_pool.tile([P, 512], f32, name="fc")
                nc.vector.tensor_mul(fc[:], q("f"), c_all[:, hb, ns])
                ig = tmp_pool.tile([P, 512], f32, name="ig")
                nc.vector.tensor_mul(ig[:], q("i"), q("g"))
                nc.vector.tensor_add(q("cn"), ig[:], fc[:])
                th = tmp_pool.tile([P, 512], f32, name="th")
                nc.scalar.activation(th[:], q("cn"), AF.Tanh)
                if final:
                    gate_act(3)  # o

                if final and hb == HB - 1:
                    # g,i,f close early and cn/tanh finish during the o
                    # matmul block: stream those out before the last MM so
                    # the post-MM tail is just o-act -> h-mul halves ->
                    # small parallel stores (and their HBM receipts overlap
                    # an otherwise-quiet HBM).
                    nc.sync.dma_start(out_r[gi][:, 0 : 3 * 512], ot[:, 0 : 3 * 512])
                    nc.scalar.dma_start(
                        out_r[gi][:, 3 * 512 : 4 * 512], ot[:, 3 * 512 : 4 * 512]
                    )
                    nc.scalar.dma_start(
                        out_r[gi][:, 4 * 512 : 5 * 512], ot[:, 4 * 512 : 5 * 512]
                    )
                    h0 = 5 * 512
                    engs = (nc.sync, nc.scalar)
                    for half in range(2):
                        hs = slice(h0 + half * 256, h0 + (half + 1) * 256)
                        ts_ = slice(half * 256, (half + 1) * 256)
                        nc.vector.tensor_mul(ot[:, hs], ot[:, 4 * 512 + half * 256 : 4 * 512 + (half + 1) * 256], th[:, ts_])
                        engs[half].dma_start(out_r[gi][:, hs], ot[:, hs])
                else:
                    nc.vector.tensor_mul(q("h"), q("o"), th[:])
                    nc.sync.dma_start(out_r[gi][:, :], ot[:])

    nc.compile()
    return nc


def _get_program():
    if "nc" not in _CACHE:
        _CACHE["nc"] = _build_program()
    return _CACHE["nc"]


def _gate_row_index(core: int) -> np.ndarray:
    """Global rows of W/b (4H-dim) owned by `core`, in [hb][gate][r] order."""
    idx = []
    for hb in range(HB):
        for g in range(4):
            base = g * H + core * S + hb * P
            idx.extend(range(base, base + P))
    return np.asarray(idx)


def _part_major(a2d: np.ndarray) -> np.ndarray:
    """[K, M] -> [(p a), M] rows ordered p-major (row = p*KC + a)."""
    k, m = a2d.shape
    assert k == P * KC
    return np.ascontiguousarray(
        a2d.reshape(KC, P, m).transpose(1, 0, 2).reshape(k, m)
    )


def kernel(x, h_prev, C_prev, W_ih, b_ih, W_hh, b_hh):
    from concourse.bass_utils import run_bass_kernel_spmd

    nc = _get_program()

    xh_full = np.concatenate([x, h_prev], axis=1).T.astype(_BF16)  # [4096, 2048]
    xh_gens = [
        _part_major(np.ascontiguousarray(xh_full[:, n * 512 : (n + 1) * 512]))
        for n in range(NT)
    ]
    bias_full = (b_ih + b_hh).astype(np.float32)

    in_maps = []
    for c in range(NCORES):
        idx = _gate_row_index(c)
        w_cat = np.concatenate([W_ih[idx], W_hh[idx]], axis=1).astype(_BF16)
        cs = C_prev[:, c * S : (c + 1) * S].T  # [256, 2048]
        in_map = {
            "w_t": _part_major(np.ascontiguousarray(w_cat.T)),  # [4096, 1024]
            "bias": np.ascontiguousarray(bias_full[idx].reshape(4 * HB, P).T),
            "c_t": np.ascontiguousarray(
                cs.reshape(HB, P, B).transpose(1, 0, 2).reshape(HB * P, B)
            ),
        }
        for n in range(NT):
            in_map[f"xh{n}"] = xh_gens[n]
        in_maps.append(in_map)

    _CACHE["last_in_maps"] = in_maps
    res = run_bass_kernel_spmd(nc, in_maps, core_ids=list(range(NCORES)))

    # res.results[c]["out"]: [8*128, 3072] -> [n, hb, p, q, col]
    QNAMES = ["g_t", "i_t", "f_t", "cn_t", "o_t", "h_t"]
    full = {}
    parts = [
        res.results[c]["out"].reshape(NT, HB, P, NQ, 512) for c in range(NCORES)
    ]
    for qi, qn in enumerate(QNAMES):
        # rows: core-major h index (c, hb, p); cols: (n, col)
        t = np.concatenate(
            [
                parts[c][:, :, :, qi, :]
                .transpose(1, 2, 0, 3)
                .reshape(S, B)
                for c in range(NCORES)
            ],
            axis=0,
        )  # [H, B]
        full[qn] = np.ascontiguousarray(t.T)

    return (
        full["h_t"],
        full["cn_t"],
        full["f_t"],
        full["i_t"],
        full["g_t"],
        full["o_t"],
    )


# revision 13
# speedup vs baseline: 1.1925x; 1.1708x over previous
"""TRN2 Bass kernel: fused LSTM cell (nn_CustomLSTMCell), 8-core tensor-parallel.

Strategy
--------
gates = x @ W_ih.T + b_ih + h_prev @ W_hh.T + b_hh  is computed as ONE GEMM
with contraction K = I + H = 4096 over xh = [x | h_prev] and W = [W_ih | W_hh].

The 4H gate dimension is tensor-parallel sharded across the 8 cores: core c
owns h-columns [c*256, (c+1)*256) of every gate (i, f, g, o).  Each core
computes gatesT [1024, 2048] = Wc @ xh.T with gate rows on partitions, so the
per-gate bias is a native per-partition scalar in scalar.activation, which
also applies sigmoid/tanh while evicting PSUM -> SBUF.  The LSTM cell update
(new_C = f*C + i*g, new_h = o*tanh(new_C)) runs on the vector engine, fully
overlapped with the tensor engine.  No collectives: output slices are
disjoint and gathered on the host.

Matmul operands are cast to fp16 on the host (halves DMA traffic, 4x PE rate
vs fp32, ~8x more mantissa than bf16); accumulation stays fp32 in PSUM and
the epilogue is fp32.

Loop shape: all 8 PSUM banks (2 h-blocks x 4 gates) accumulate
simultaneously with the k-chunk loop OUTER, so one 384KB chunk of
(w, xh) feeds 8 matmuls -> the DMA demand rate of the first batch-tile is
~220GB/s, below what the DMA rings sustain (~2 HWDGE rings x ~170GB/s);
the old per-h-block loop demanded ~300GB/s and starved the PE early, which
also dropped the HAM clock to 1.2GHz for several us.  The last few chunks
run per-bank (staggered tail) so PSUM evictions pipeline and the next
group's bank reuse never stalls.

DMA plan: few, large transfers with multi-KB contiguous per-partition lines
(host pre-arranges every tensor partition-major).  w streams in 1MB blocks
alternating between the Sync HWDGE and GpSimd SWDGE queues, xh generation 0
on the Scalar HWDGE queue, later generations on whichever ring is free, and
each group's six outputs are packed into one [128, 3072] tile stored with a
single DMA.
"""

import numpy as np

B = 2048           # batch
I_DIM = 2048       # input features
H = 2048           # hidden
NCORES = 8
S = H // NCORES    # 256: per-core h-slice (per gate)
M_PER_CORE = 4 * S # 1024 gate rows per core
K = I_DIM + H      # 4096 fused contraction dim
P = 128
KC = K // P        # 32 contraction chunks
NT = B // 512      # 4 batch tiles of 512
HB = S // P        # 2 h-blocks of 128 per core
NQ = 6             # packed outputs per group: f,i,g,cn,o,h

_BF16 = np.float16

_CACHE = {}

# k-chunk DMA blocks, in consumption order on a single ring each: first
# block small so the PE's first real matmul starts early; later blocks big
# for DMA efficiency.  Sized so each block completes (at the ~170GB/s one
# ring sustains when two rings are active) >=3us before the PE needs its
# first chunk at the fused-loop rate of 1.73us/chunk.
_XBLOCKS = [(0, 2), (2, 8), (8, 16), (16, 32)]
_WBLOCKS = [(0, 2), (2, 6), (6, 12), (12, 20), (20, 32)]


def _build_program():
    from contextlib import ExitStack

    import concourse.mybir as mybir
    import concourse.tile as tile
    from concourse import bacc

    f32 = mybir.dt.float32
    bf16 = mybir.dt.float16
    AF = mybir.ActivationFunctionType

    nc = bacc.Bacc("TRN2", target_bir_lowering=False, debug=False)

    # Host layouts are partition-major so every DMA moves multi-KB
    # contiguous lines per partition.
    w_d = nc.dram_tensor("w_t", [P * KC, 8 * P], bf16, kind="ExternalInput").ap()
    xh_d = [
        nc.dram_tensor(f"xh{n}", [P * KC, 512], bf16, kind="ExternalInput").ap()
        for n in range(NT)
    ]
    bias_d = nc.dram_tensor("bias", [P, 4 * HB], f32, kind="ExternalInput").ap()
    c_d = nc.dram_tensor("c_t", [P * HB, B], f32, kind="ExternalInput").ap()
    out_d = nc.dram_tensor(
        "out", [NT * HB * P, NQ * 512], f32, kind="ExternalOutput"
    ).ap()

    w_r = w_d.rearrange("(p a) m -> p a m", p=P)         # [128, 32, 1024]
    xh_r = [t.rearrange("(p a) m -> p a m", p=P) for t in xh_d]
    c_r = c_d.rearrange("(p h) m -> p h m", p=P)         # [128, 2, 2048]
    out_r = out_d.rearrange("(g p) m -> g p m", g=NT * HB)  # [8, 128, 3072]

    # gate order within a group: i, f, g, o (torch chunk order); packed
    # output column order: g, i, f, cn, o, h (early finishers first so the
    # final group's tail stores can fire before o/h are ready).
    ACT_FN = [AF.Sigmoid, AF.Sigmoid, AF.Tanh, AF.Sigmoid]
    QCOL = {"g": 0, "i": 1, "f": 2, "cn": 3, "o": 4, "h": 5}
    QN = ["i", "f", "g", "o"]

    with tile.TileContext(nc) as tc, ExitStack() as ctx:
        w_pool = ctx.enter_context(tc.tile_pool(name="w", bufs=1))
        xh_pool = ctx.enter_context(tc.tile_pool(name="xh", bufs=2))
        c_pool = ctx.enter_context(tc.tile_pool(name="c", bufs=1))
        b_pool = ctx.enter_context(tc.tile_pool(name="b", bufs=1))
        psum_pool = ctx.enter_context(tc.tile_pool(name="ps", bufs=1, space="PSUM"))
        out_pool = ctx.enter_context(tc.tile_pool(name="out", bufs=2))
        tmp_pool = ctx.enter_context(tc.tile_pool(name="tmp", bufs=2))

        # Bias rides the (otherwise idle-early) GpSimd queue so it never
        # delays the critical w/xh streams; it's only needed at ~60us.
        bias_all = b_pool.tile([P, 4 * HB], f32)
        nc.gpsimd.dma_start(bias_all[:], bias_d[:, :])

        # A few matmuls on dummy data bridge the framework-preamble gap so
        # HAM warm-up overlaps the first block DMAs.  Never read.
        dummy = b_pool.tile([P, 512], bf16)
        nc.vector.memset(dummy[:], 0.0)
        warm_ps = psum_pool.tile([P, 512], f32, name="ps00")
        NWARM = 10
        for i in range(NWARM):
            nc.tensor.matmul(
                warm_ps[:], dummy[:, 0:P], dummy[:],
                start=(i == 0), stop=(i == NWARM - 1),
            )

        # Input streams, one ring per stream, each in strict consumption
        # order (concurrent rings split the ~340GB/s fair-share by packet,
        # so putting later-needed data on the ring ahead of earlier-needed
        # data starves the PE).  Sync: w blocks ascending.  Scalar: xh0,
        # then C (first epilogue ~60us), then xh1 (group 1 starts ~66us).
        # GpSimd picks up xh2/xh3 inside the loop (their buffer-reuse waits
        # must not block a queue with timely work).
        w_sb = w_pool.tile([P, KC, 8 * P], bf16)
        for a0, a1 in _WBLOCKS:
            nc.sync.dma_start(w_sb[:, a0:a1, :], w_r[:, a0:a1, :])

        def alloc_xh():
            return xh_pool.tile([P, KC, 512], bf16, name="xhg")

        xh_tiles = {0: alloc_xh(), 1: alloc_xh()}
        for a0, a1 in _XBLOCKS:
            nc.scalar.dma_start(xh_tiles[0][:, a0:a1, :], xh_r[0][:, a0:a1, :])

        c_all = c_pool.tile([P, HB, B], f32)
        nc.scalar.dma_start(c_all[:], c_r[:, :, :])

        nc.scalar.dma_start(xh_tiles[1][:, 0:16, :], xh_r[1][:, 0:16, :])
        nc.scalar.dma_start(xh_tiles[1][:, 16:KC, :], xh_r[1][:, 16:KC, :])

        def mm(ps_t, hb, g, k, **kw):
            m0 = (hb * 4 + g) * P
            nc.tensor.matmul(
                ps_t[:], w_sb[:, k, m0 : m0 + P], xh[:, k, :], **kw
            )

        for n in range(NT):
            ns = slice(n * 512, (n + 1) * 512)
            final = n == NT - 1
            # prefetch the n+1 generation one iteration ahead on GpSimd
            # (slot frees when the n-1 generation's readers finish)
            if n >= 1 and n + 1 < NT:
                xh_tiles[n + 1] = alloc_xh()
                nc.gpsimd.dma_start(
                    xh_tiles[n + 1][:, 0:16, :], xh_r[n + 1][:, 0:16, :]
                )
                nc.gpsimd.dma_start(
                    xh_tiles[n + 1][:, 16:KC, :], xh_r[n + 1][:, 16:KC, :]
                )
            xh = xh_tiles[n]

            ps = {
                (hb, g): psum_pool.tile([P, 512], f32, name=f"ps{hb}{g}")
                for hb in range(HB)
                for g in range(4)
            }
            # per-bank split point between the interleaved phase and the
            # bank's contiguous tail.  The very last bank (hb1's o) gets a
            # 12-chunk tail so f closes ~12 MMs before the end and the
            # cn/tanh chain finishes before the last matmul.
            splits = {
                (hb, g): (
                    KC - 4 if not final
                    else KC - 12 if (g == 3 and hb == HB - 1)
                    else KC - 8
                )
                for hb in range(HB)
                for g in range(4)
            }
            for k in range(KC):
                for hb in range(HB):
                    for g in range(4):
                        if k < splits[hb, g]:
                            mm(ps[hb, g], hb, g, k, start=(k == 0), stop=False)

            # staggered tail: each bank finishes its last chunks as a
            # contiguous run, so evictions (and the epilogues) pipeline
            # against the remaining banks' matmuls.  For the final group
            # the g,i,f banks of hb1 close before o (g first: the i*g and
            # f*C products gate the tanh), so the whole cell-state chain
            # completes during the o matmuls and the post-MM tail is just
            # o-sigmoid -> h-mul -> store.
            for hb in range(HB):
                gate_order = (2, 0, 1, 3) if final else (0, 1, 2, 3)
                gi = n * HB + hb
                ot = out_pool.tile([P, NQ * 512], f32, name="og")

                def q(name):
                    j = QCOL[name]
                    return ot[:, j * 512 : (j + 1) * 512]

                def gate_act(g):
                    m = hb * 4 + g
                    nc.scalar.activation(
                        q(QN[g]), ps[hb, g][:], ACT_FN[g],
                        bias=bias_all[:, m : m + 1],
                    )

                last_g = gate_order[3]
                for g in gate_order[:3]:
                    for k in range(splits[hb, g], KC):
                        mm(ps[hb, g], hb, g, k, start=False, stop=(k == KC - 1))
                if final:
                    # acts g,i,f and the cell-state chain overlap the o
                    # matmul tail; only o-sigmoid + h-mul remain after it.
                    for g in gate_order[:3]:
                        gate_act(g)
                    for k in range(splits[hb, last_g], KC):
                        mm(ps[hb, last_g], hb, last_g, k,
                           start=False, stop=(k == KC - 1))
                else:
                    # o-act is emitted before the (cn-gated) tanh so the
                    # o PSUM bank frees promptly for the next group.
                    for k in range(splits[hb, last_g], KC):
                        mm(ps[hb, last_g], hb, last_g, k,
                           start=False, stop=(k == KC - 1))
                    for g in gate_order[:3]:
                        gate_act(g)
                    gate_act(3)

                fc = tmp_pool.tile([P, 512], f32, name="fc")
                nc.vector.tensor_mul(fc[:], q("f"), c_all[:, hb, ns])
                ig = tmp_pool.tile([P, 512], f32, name="ig")
                nc.vector.tensor_mul(ig[:], q("i"), q("g"))
                nc.vector.tensor_add(q("cn"), ig[:], fc[:])
                th = tmp_pool.tile([P, 512], f32, name="th")
                nc.scalar.activation(th[:], q("cn"), AF.Tanh)
                if final:
                    gate_act(3)  # o

                if final and hb == HB - 1:
                    # g,i,f close early and cn/tanh finish during the o
                    # matmul block: stream those out before the last MM so
                    # the post-MM tail is just o-act -> h-mul halves ->
                    # small parallel stores (and their HBM receipts overlap
                    # an otherwise-quiet HBM).
                    nc.sync.dma_start(out_r[gi][:, 0 : 3 * 512], ot[:, 0 : 3 * 512])
                    nc.scalar.dma_start(
                        out_r[gi][:, 3 * 512 : 4 * 512], ot[:, 3 * 512 : 4 * 512]
                    )
                    nc.scalar.dma_start(
                        out_r[gi][:, 4 * 512 : 5 * 512], ot[:, 4 * 512 : 5 * 512]
                    )
                    h0 = 5 * 512
                    engs = (nc.sync, nc.scalar)
                    for half in range(2):
                        hs = slice(h0 + half * 256, h0 + (half + 1) * 256)
                        ts_ = slice(half * 256, (half + 1) * 256)
                        nc.vector.tensor_mul(ot[:, hs], ot[:, 4 * 512 + half * 256 : 4 * 512 + (half + 1) * 256], th[:, ts_])
                        engs[half].dma_start(out_r[gi][:, hs], ot[:, hs])
                else:
                    nc.vector.tensor_mul(q("h"), q("o"), th[:])
                    nc.sync.dma_start(out_r[gi][:, :], ot[:])

    nc.compile()
    return nc


def _get_program():
    if "nc" not in _CACHE:
        _CACHE["nc"] = _build_program()
    return _CACHE["nc"]


def _gate_row_index(core: int) -> np.ndarray:
    """Global rows of W/b (4H-dim) owned by `core`, in [hb][gate][r] order."""
    idx = []
    for hb in range(HB):
        for g in range(4):
            base = g * H + core * S + hb * P
            idx.extend(range(base, base + P))
    return np.asarray(idx)


def _part_major(a2d: np.ndarray) -> np.ndarray:
    """[K, M] -> [(p a), M] rows ordered p-major (row = p*KC + a)."""
    k, m = a2d.shape
    assert k == P * KC
    return np.ascontiguousarray(
        a2d.reshape(KC, P, m).transpose(1, 0, 2).reshape(k, m)
    )


def kernel(x, h_prev, C_prev, W_ih, b_ih, W_hh, b_hh):
    from concourse.bass_utils import run_bass_kernel_spmd

    nc = _get_program()

    xh_full = np.concatenate([x, h_prev], axis=1).T.astype(_BF16)  # [4096, 2048]
    xh_gens = [
        _part_major(np.ascontiguousarray(xh_full[:, n * 512 : (n + 1) * 512]))
        for n in range(NT)
    ]
    bias_full = (b_ih + b_hh).astype(np.float32)

    in_maps = []
    for c in range(NCORES):
        idx = _gate_row_index(c)
        w_cat = np.concatenate([W_ih[idx], W_hh[idx]], axis=1).astype(_BF16)
        cs = C_prev[:, c * S : (c + 1) * S].T  # [256, 2048]
        in_map = {
            "w_t": _part_major(np.ascontiguousarray(w_cat.T)),  # [4096, 1024]
            "bias": np.ascontiguousarray(bias_full[idx].reshape(4 * HB, P).T),
            "c_t": np.ascontiguousarray(
                cs.reshape(HB, P, B).transpose(1, 0, 2).reshape(HB * P, B)
            ),
        }
        for n in range(NT):
            in_map[f"xh{n}"] = xh_gens[n]
        in_maps.append(in_map)

    _CACHE["last_in_maps"] = in_maps
    res = run_bass_kernel_spmd(nc, in_maps, core_ids=list(range(NCORES)))

    # res.results[c]["out"]: [8*128, 3072] -> [n, hb, p, q, col]
    QNAMES = ["g_t", "i_t", "f_t", "cn_t", "o_t", "h_t"]
    full = {}
    parts = [
        res.results[c]["out"].reshape(NT, HB, P, NQ, 512) for c in range(NCORES)
    ]
    for qi, qn in enumerate(QNAMES):
        # rows: core-major h index (c, hb, p); cols: (n, col)
        t = np.concatenate(
            [
                parts[c][:, :, :, qi, :]
                .transpose(1, 2, 0, 3)
                .reshape(S, B)
                for c in range(NCORES)
            ],
            axis=0,
        )  # [H, B]
        full[qn] = np.ascontiguousarray(t.T)

    return (
        full["h_t"],
        full["cn_t"],
        full["f_t"],
        full["i_t"],
        full["g_t"],
        full["o_t"],
    )
